# revision 10
# baseline (speedup 1.0000x reference)
"""DigitCapsule dynamic-routing kernel for 8 TRN2 NeuronCores.

Strategy: the reference routing is fully independent per output capsule c
(softmax over routes, sums over routes, batch-mean are all per-c). So we
shard the C=64 capsules 8-ways: each core gets W[:, 8k:8k+8] and a
replicated x. Zero collectives; identical SPMD program per core with
per-core inputs.

Per core (B=64, R=2048, I=8, CL=8, O=16; K-dim = (r,i) = 16384 = 128
k-tiles of 128 = (16 routes q, 8 i)). s/v tensors live as
[b=64, (o,c)=128]; routing state lives banded as [(j,q)=128, (g,lo,c)].

  pass 0:  n0[b,(o,c)] = sum_t xt_t^T @ wk_t          (c_ij uniform)
           v = n|n| / (R^2 + n^2)       == squash(n/R), exact algebra
  iter 1,2 (phased so each engine gets long dense runs):
    A: G[(q,i),(lo,(o,c))] = xn^T @ V for all 128 k-tiles — fp8 xn as
       stationary, row-pair tiled (two concurrent 64-row matmuls);
       per block: ACT drains PSUM->bf16, P = G (.) Wr (DVE/GPS)
    B: BD-matmul bands psb[(j,q),(lo,o,c)] per grp; ored = reduce_o;
       bstate += ored/B; wexpb = exp(bstate); wrep matmuls interleaved
    D: WW = Wr (.) wrep (broadcast o);  n += xt_t^T @ WW_t
    Z[c] = sum_r wexp;  v = n|n| / (Z^2 + n^2)  == squash(n/Z), exact
  out[b,(o,c)] = v (f32)
"""

import os
import sys

for _p in ("/opt/trn_rl_repo", "/root/.axon_site/_ro/trn_rl_repo"):
    if os.path.isdir(_p) and _p not in sys.path:
        sys.path.insert(0, _p)

from contextlib import ExitStack

import numpy as np

import concourse.bass as bass
import concourse.bacc as bacc
from concourse import mybir
from concourse.bass_utils import run_bass_kernel_spmd
from concourse.tile import TileContext

B, R, C, O, I = 64, 2048, 64, 16, 8
N_CORES = 8
CL = C // N_CORES            # capsules per core = 8
F = CL * O                   # free (o,c) = 128
NT = R // 16                 # 128 k-tiles; tile t = routes [16t,16t+16), part p=(q,i)
NB = 16                      # number of 8-k-tile blocks
BLK = NT // NB               # 8 k-tiles per block

# which of the 16 P / WW multiplies per iter go to GpSimd instead of DVE
GPS_P = int(os.environ.get("CAPS_GPS_P", "3"))
GPS_WW = int(os.environ.get("CAPS_GPS_WW", "2"))
P_GPS_SET = {3, 7, 11, 15}  # last block of each grp (most slack before BD j=3)
WW_GPS_SET = {15, 14, 13, 12}  # last consumers in the n-matmul sequence
# blocks whose P is multiplied straight from PSUM on DVE (skip ACT drain)
DIRECT = int(os.environ.get("CAPS_DIRECT", "4"))
DIRECT_SET = {1, 5, 9, 13}
# HAM warmup dummy matmuls (N=512) per burst
WARM_BOUND = int(os.environ.get("CAPS_WARM_BOUND", "12"))
WARM_PASS0 = int(os.environ.get("CAPS_WARM_PASS0", "6"))


def _consts_np():
    """cstb [128,1024] bf16: BDF4 [0:512), BDT [512:1024).
    cstf [128,65] f32: masked-ones col 0; ones-row (partition 0) cols [1:65)."""
    cstb = np.zeros((128, 1024), dtype=np.float32)
    p = np.arange(128)
    # BDF4_j[p=(q,i), m] = 1 iff m == 32j + p//8  (i-reduce into band 32j+q)
    for j in range(4):
        cstb[p, 128 * j + 32 * j + p // 8] = 1.0
    # BDT_j = BDF4_j^T (band (j,q) -> rows (q,i))
    for j in range(4):
        cstb[:, 512 + 128 * j:512 + 128 * (j + 1)] = \
            cstb[:, 128 * j:128 * (j + 1)].T
    cstf = np.zeros((128, 65), dtype=np.float32)
    # Z-reduce mask: only band rows 32j+q (q<16) hold real data; the other
    # 64 partitions of wexpb are exp(0)=1 junk and must not enter Z.
    cstf[p[(p % 32) < 16], 0] = 1.0
    cstf[0, 1:65] = 1.0
    return cstb, cstf


def build_bass():
    f32 = mybir.dt.float32
    cdt = mybir.dt.bfloat16
    f8 = mybir.dt.float8e4

    nc = bacc.Bacc()
    # wxt: 8 chunks of [wk 2048 | xt 1024] columns
    wxt_d = nc.declare_dram_parameter("wxt", [128, 8 * 3072], cdt, isOutput=False)
    # xn8: fp8 x, natural layout on partitions 0:64
    # xn8: fp8 x, natural layout on partitions 0:64
    xn8_d = nc.declare_dram_parameter("xn8", [64, NT * 128], f8, isOutput=False)
    cstb_d = nc.declare_dram_parameter("cstb", [128, 1024], cdt, isOutput=False)
    cstf_d = nc.declare_dram_parameter("cstf", [128, 65], f32, isOutput=False)
    out_d = nc.declare_dram_parameter("out", [B, F], f32, isOutput=True)

    with TileContext(nc) as tc, ExitStack() as ctx:
        big = ctx.enter_context(tc.tile_pool(name="big", bufs=1))
        small = ctx.enter_context(tc.tile_pool(name="small", bufs=3))
        pgpool = ctx.enter_context(tc.tile_pool(name="pgpool", bufs=3))
        p16 = ctx.enter_context(tc.tile_pool(name="p16", bufs=NB + 1))
        wwpool = ctx.enter_context(tc.tile_pool(name="wwpool", bufs=4))
        ps_acc = ctx.enter_context(tc.tile_pool(name="ps_acc", bufs=1, space="PSUM"))
        ps_gb = ctx.enter_context(tc.tile_pool(name="ps_gb", bufs=3, space="PSUM"))
        ps_misc = ctx.enter_context(tc.tile_pool(name="ps_misc", bufs=1, space="PSUM"))

        # ---- load inputs (consts first: small and needed early) ----
        cstb = big.tile([128, 1024], cdt, tag="cstb", name="cstb")
        nc.sync.dma_start(out=cstb, in_=cstb_d[:])
        cstf = big.tile([128, 65], f32, tag="cstf", name="cstf")
        nc.sync.dma_start(out=cstf, in_=cstf_d[:])
        wxt = [big.tile([128, 3072], cdt, tag=f"wxt{h}", name=f"wxt{h}")
               for h in range(8)]
        for h in range(8):
            nc.sync.dma_start(out=wxt[h], in_=wxt_d[:, h * 3072:(h + 1) * 3072])
        xn8 = big.tile([64, NT * 128], f8, tag="xn8", name="xn8")
        for piece in range(2):
            c0 = piece * 8192
            nc.sync.dma_start(out=xn8[:, c0:c0 + 8192],
                              in_=xn8_d[:, c0:c0 + 8192])

        BDF4 = cstb[:, 0:512]
        BDT = cstb[:, 512:1024]
        onesm = cstf[:, 0:1]
        onesrow = cstf[0:1, 1:65]

        def wk_tile(t):
            h, lo = t // 16, t % 16
            return wxt[h][:, lo * 128:(lo + 1) * 128]

        def xt_tile(t):
            h, lo = t // 16, t % 16
            return wxt[h][:, 2048 + lo * 64:2048 + (lo + 1) * 64]

        def wk_block(hb):
            # [128, 8, 128] view of block hb's 8 k-tiles of W
            wkh = wxt[hb // 2][:, 0:2048].rearrange("p (u f) -> p u f", f=128)
            return wkh[:, (hb % 2) * BLK:(hb % 2) * BLK + BLK, :]

        # v = n*|n| / (zsq + n^2); returns V bf16 (mk_V) or out f32
        def squash_from(ps_n, zsq_sb, mk_V):
            absn = small.tile([64, 128], f32, tag="absn", name="absn")
            nc.scalar.activation(absn, ps_n, mybir.ActivationFunctionType.Abs)
            nsq = small.tile([64, 128], f32, tag="nsq", name="nsq")
            nc.scalar.activation(nsq, ps_n, mybir.ActivationFunctionType.Square)
            den = small.tile([64, 128], f32, tag="den", name="den")
            if zsq_sb is None:
                nc.vector.tensor_scalar_add(den, nsq, float(R) * float(R))
            else:
                nc.vector.tensor_add(den, nsq, zsq_sb)
            rden = small.tile([64, 128], f32, tag="rden", name="rden")
            nc.vector.reciprocal_approx_fast(rden, den)
            num = small.tile([64, 128], f32, tag="num", name="num")
            nc.vector.tensor_mul(num, ps_n, absn)
            if not mk_V:
                out_sb = small.tile([64, 128], f32, tag="outsb", name="outsb")
                nc.vector.tensor_mul(out_sb, num, rden)
                return out_sb
            v64 = small.tile([64, 128], cdt, tag="V", name="V", bufs=2)
            nc.vector.tensor_mul(v64, num, rden)
            return v64

        # HAM warmup: long-stream dummy matmuls into a scratch PSUM tile to
        # keep the PE array's activity monitor at full clock across stalls.
        def warm(n, rhs):
            if n <= 0:
                return
            dmy = ps_gb.tile([128, 512], f32, tag="gb", name="warm")
            for _ in range(n):
                nc.tensor.matmul(dmy, lhsT=cstb[:, 0:128], rhs=rhs,
                                 start=True, stop=True)

        # ---- pass 0: n0 = sum_t xt_t^T @ wk_t ; V = squash ----
        warm(8, cstb[:, 0:512])
        ps_s = ps_acc.tile([64, 128], f32, tag="acc", name="acc")
        for t in range(NT):
            nc.tensor.matmul(ps_s, lhsT=xt_tile(t), rhs=wk_tile(t),
                             start=(t == 0), stop=(t == NT - 1))
            if t % 16 == 15 and t // 16 < 7:
                warm(WARM_PASS0, wxt[t // 16][:, 0:512])
        V = squash_from(ps_s, None, True)
        warm(WARM_BOUND, wxt[7][:, 0:512])

        bstate = small.tile([128, 256], f32, tag="bstate", name="bstate", bufs=1)
        nc.vector.memset(bstate, 0.0)
        wexpb = small.tile([128, 256], cdt, tag="wexpb", name="wexpb", bufs=1)

        for it in (1, 2):
            ps_n = ps_acc.tile([64, 128], f32, tag="acc", name="acc")
            Ps = [None] * NB
            # ---------- phase A: all G matmuls (fp8 stationary x) ----------
            for hb in range(NB):
                psg = ps_gb.tile([128, BLK * 128], f32, tag="gb", name="gb")
                for lo in range(BLK):
                    t = hb * BLK + lo
                    nc.tensor.matmul(
                        psg[:, lo * 128:(lo + 1) * 128],
                        lhsT=xn8[:, t * 128:(t + 1) * 128], rhs=V,
                        start=True, stop=True,
                    )
                P = p16.tile([128, BLK * 128], cdt, tag="P", name="P")
                if hb in DIRECT_SET and (hb - 1) // 4 < DIRECT:
                    nc.vector.tensor_tensor(
                        P.rearrange("p (u f) -> p u f", f=128),
                        psg.rearrange("p (u f) -> p u f", f=128),
                        wk_block(hb),
                        op=mybir.AluOpType.mult,
                    )
                else:
                    Pg = pgpool.tile([128, BLK * 128], cdt, tag="Pg", name="Pg")
                    nc.scalar.activation(Pg, psg,
                                         mybir.ActivationFunctionType.Copy)
                    eng = nc.gpsimd if (hb in P_GPS_SET and
                                        len(P_GPS_SET) - list(sorted(P_GPS_SET)).index(hb) <= GPS_P) \
                        else nc.vector
                    eng.tensor_tensor(
                        P.rearrange("p (u f) -> p u f", f=128),
                        Pg.rearrange("p (u f) -> p u f", f=128),
                        wk_block(hb),
                        op=mybir.AluOpType.mult,
                    )
                Ps[hb] = P

            # ---------- phase B: BD bands + b-update + wrep ----------
            psbs = [None] * 4
            wrs = [None] * NB

            def emit_bd(grp):
                psb = ps_gb.tile([128, BLK * 128], f32, tag="gb", name="gb")
                for j in range(4):
                    for half in range(2):
                        nc.tensor.matmul(
                            psb[:, half * 512:(half + 1) * 512],
                            lhsT=BDF4[:, 128 * j:128 * (j + 1)],
                            rhs=Ps[4 * grp + j][:, half * 512:(half + 1) * 512],
                            start=(j == 0), stop=(j == 3),
                        )
                psbs[grp] = psb

            def emit_bupdate(grp):
                ored = small.tile([128, 64], f32, tag="ored", name="ored",
                                  bufs=2)
                psb = psbs[grp]
                nc.vector.tensor_reduce(
                    ored.rearrange("p (l c) -> p l c", c=8),
                    bass.AP(tensor=psb.tensor, offset=psb.offset,
                            ap=[psb.ap[0], [128, 8], [1, 8], [8, 16]]),
                    axis=mybir.AxisListType.X,
                    op=mybir.AluOpType.add,
                )
                cs = slice(grp * 64, (grp + 1) * 64)
                nc.vector.scalar_tensor_tensor(bstate[:, cs], ored, 1.0 / B,
                                               bstate[:, cs],
                                               op0=mybir.AluOpType.mult,
                                               op1=mybir.AluOpType.add)
                nc.scalar.activation(wexpb[:, cs], bstate[:, cs],
                                     mybir.ActivationFunctionType.Exp)

            def emit_wrep(grp):
                cs = slice(grp * 64, (grp + 1) * 64)
                for j in range(4):
                    hb = 4 * grp + j
                    ps_wr = ps_misc.tile([128, 64], f32, tag="m", name="wrps")
                    nc.tensor.matmul(ps_wr, lhsT=BDT[:, 128 * j:128 * (j + 1)],
                                     rhs=wexpb[:, cs], start=True, stop=True)
                    wr = small.tile([128, 64], cdt, tag="wr", name="wr", bufs=5)
                    nc.scalar.activation(wr, ps_wr,
                                         mybir.ActivationFunctionType.Copy)
                    wrs[hb] = wr

            emit_bd(0)
            emit_bupdate(0)
            emit_bd(1)
            emit_bupdate(1)
            emit_wrep(0)
            emit_bd(2)
            emit_bupdate(2)
            emit_wrep(1)
            emit_bd(3)
            emit_bupdate(3)
            emit_wrep(2)
            emit_wrep(3)

            # ---------- phase D: WW + n-matmuls; Z-path in the middle ----------
            def emit_ww_n(hb):
                wr = wrs[hb]
                ww = wwpool.tile([128, BLK * 128], cdt, tag="ww", name="ww")
                in1 = bass.AP(tensor=wr.tensor, offset=wr.offset,
                              ap=[wr.ap[0], [8, 8], [0, 16], [1, 8]])
                eng = nc.gpsimd if (hb in WW_GPS_SET and
                                    list(sorted(WW_GPS_SET, reverse=True)).index(hb) < GPS_WW) \
                    else nc.vector
                eng.tensor_tensor(
                    ww.rearrange("p (l o c) -> p l o c", o=16, c=8),
                    wk_block(hb).rearrange("p l (o c) -> p l o c", c=8),
                    in1,
                    op=mybir.AluOpType.mult,
                )
                for lo in range(BLK):
                    t = hb * BLK + lo
                    nc.tensor.matmul(ps_n,
                                     lhsT=xt_tile(t),
                                     rhs=ww[:, lo * 128:(lo + 1) * 128],
                                     start=(t == 0), stop=(t == NT - 1))

            for hb in range(4):
                emit_ww_n(hb)
            # Z^2 per c, replicated to [64, 128] (overlaps n-matmuls)
            wsum = small.tile([128, 8], f32, tag="wsum", name="wsum")
            nc.vector.tensor_reduce(
                wsum,
                bass.AP(tensor=wexpb.tensor, offset=wexpb.offset,
                        ap=[wexpb.ap[0], [1, 8], [8, 32]]),
                axis=mybir.AxisListType.X, op=mybir.AluOpType.add,
            )
            ps_z = ps_misc.tile([1, 8], f32, tag="m", name="zps")
            nc.tensor.matmul(ps_z, lhsT=onesm, rhs=wsum, start=True, stop=True)
            zsq = small.tile([1, 8], f32, tag="zsq", name="zsq")
            nc.scalar.activation(zsq, ps_z, mybir.ActivationFunctionType.Square)
            zrow = small.tile([1, 128], f32, tag="zrow", name="zrow")
            nc.scalar.activation(
                zrow.rearrange("p (o c) -> p o c", c=8),
                bass.AP(tensor=zsq.tensor, offset=zsq.offset,
                        ap=[zsq.ap[0], [0, 16], [1, 8]]),
                mybir.ActivationFunctionType.Copy,
            )
            ps_zq = ps_misc.tile([64, 128], f32, tag="m", name="zqps")
            nc.tensor.matmul(ps_zq, lhsT=onesrow, rhs=zrow, start=True, stop=True)
            zqsb = small.tile([64, 128], f32, tag="zqsb", name="zqsb")
            nc.scalar.activation(zqsb, ps_zq, mybir.ActivationFunctionType.Copy)
            for hb in range(4, NB):
                emit_ww_n(hb)

            if it < 2:
                V = squash_from(ps_n, zqsb, True)
                warm(WARM_BOUND, wxt[7][:, 0:512])
            else:
                out_sb = squash_from(ps_n, zqsb, False)
                nc.sync.dma_start(out=out_d[:], in_=out_sb)

    nc.finalize()
    return nc


def _host_prep(x, W):
    """Build per-core input dicts."""
    import ml_dtypes
    ct = ml_dtypes.bfloat16
    f8 = ml_dtypes.float8_e4m3fn
    x = np.ascontiguousarray(x, dtype=np.float32)
    W = np.ascontiguousarray(W, dtype=np.float32)
    # xt[p=(q,i), t*64+b] = x[b, 16t+q, i]
    xt = x.reshape(B, NT, 16, I).transpose(2, 3, 1, 0).reshape(128, NT, 64)
    # xn8[b, t*128 + q*8+i] = x[b, 16t+q, i]  (fp8, partitions 0..63)
    xn8 = x.reshape(B, NT * 128)
    cstb, cstf = _consts_np()
    in_maps = []
    for k in range(N_CORES):
        Ws = W[:, k * CL:(k + 1) * CL]  # [R, 8, O, I]
        wk = (Ws.reshape(NT, 16, CL, O, I).transpose(1, 4, 0, 3, 2)
              .reshape(128, NT, 128))
        wxt = np.zeros((128, 8, 3072), dtype=np.float32)
        for h in range(8):
            wxt[:, h, 0:2048] = wk[:, 16 * h:16 * (h + 1), :].reshape(128, 2048)
            wxt[:, h, 2048:3072] = xt[:, 16 * h:16 * (h + 1), :].reshape(128, 1024)
        in_maps.append({
            "wxt": np.ascontiguousarray(wxt.reshape(128, 8 * 3072), dtype=ct),
            "xn8": xn8.astype(f8),
            "cstb": cstb.astype(ct),
            "cstf": cstf,
        })
    return in_maps


_CACHE = {}


def _get_nc():
    if "nc" not in _CACHE:
        _CACHE["nc"] = build_bass()
    return _CACHE["nc"]


def run(x, W, trace=False):
    nc = _get_nc()
    in_maps = _host_prep(x, W)
    res = run_bass_kernel_spmd(nc, in_maps, core_ids=list(range(N_CORES)),
                               trace=trace)
    outs = [np.asarray(res.results[k]["out"], dtype=np.float32)
            for k in range(N_CORES)]
    # out[b, (o, c)]: core k holds capsules [8k, 8k+8)
    v = np.concatenate(
        [o.reshape(B, O, CL).transpose(0, 2, 1) for o in outs], axis=1)
    return v[..., None], res


def kernel(x, W):
    v, _ = run(np.asarray(x), np.asarray(W))
    return v


# revision 11
# speedup vs baseline: 1.0332x; 1.0332x over previous
"""DigitCapsule dynamic-routing kernel for 8 TRN2 NeuronCores.

Strategy: the reference routing is fully independent per output capsule c
(softmax over routes, sums over routes, batch-mean are all per-c). So we
shard the C=64 capsules 8-ways: each core gets W[:, 8k:8k+8] and a
replicated x. Zero collectives; identical SPMD program per core with
per-core inputs.

Per core (B=64, R=2048, I=8, CL=8, O=16; K-dim = (r,i) = 16384 = 128
k-tiles of 128 = (16 routes q, 8 i)). s/v tensors live as
[b=64, (o,c)=128]; routing state lives banded as [(j,q)=128, (g,lo,c)].

  pass 0:  n0[b,(o,c)] = sum_t xt_t^T @ wk_t          (c_ij uniform)
           v = n|n| / (R^2 + n^2)       == squash(n/R), exact algebra
  iter 1,2 (phased so each engine gets long dense runs):
    A: G[(q,i),(lo,(o,c))] = xn^T @ V for all 128 k-tiles — fp8 xn as
       stationary, row-pair tiled (two concurrent 64-row matmuls);
       per block: ACT drains PSUM->bf16, P = G (.) Wr (DVE/GPS)
    B: BD-matmul bands psb[(j,q),(lo,o,c)] per grp; ored = reduce_o;
       bstate += ored/B; wexpb = exp(bstate); wrep matmuls interleaved
    D: WW = Wr (.) wrep (broadcast o);  n += xt_t^T @ WW_t
    Z[c] = sum_r wexp;  v = n|n| / (Z^2 + n^2)  == squash(n/Z), exact
  out[b,(o,c)] = v (f32)
"""

import os
import sys

for _p in ("/opt/trn_rl_repo", "/root/.axon_site/_ro/trn_rl_repo"):
    if os.path.isdir(_p) and _p not in sys.path:
        sys.path.insert(0, _p)

from contextlib import ExitStack

import numpy as np

import concourse.bass as bass
import concourse.bacc as bacc
from concourse import mybir
from concourse.bass_utils import run_bass_kernel_spmd
from concourse.tile import TileContext

B, R, C, O, I = 64, 2048, 64, 16, 8
N_CORES = 8
CL = C // N_CORES            # capsules per core = 8
F = CL * O                   # free (o,c) = 128
NT = R // 16                 # 128 k-tiles; tile t = routes [16t,16t+16), part p=(q,i)
NB = 16                      # number of 8-k-tile blocks
BLK = NT // NB               # 8 k-tiles per block

# which of the 16 P / WW multiplies per iter go to GpSimd instead of DVE
GPS_P = int(os.environ.get("CAPS_GPS_P", "3"))
GPS_WW = int(os.environ.get("CAPS_GPS_WW", "2"))
P_GPS_SET = {3, 7, 11, 15}  # last block of each grp (most slack before BD j=3)
WW_GPS_SET = {15, 14, 13, 12}  # last consumers in the n-matmul sequence
# blocks whose P is multiplied straight from PSUM on DVE (skip ACT drain)
DIRECT = int(os.environ.get("CAPS_DIRECT", "4"))
DIRECT_SET = {1, 5, 9, 13}
# HAM warmup dummy matmuls (N=512) per burst
WARM_BOUND = int(os.environ.get("CAPS_WARM_BOUND", "8"))
WARM_PASS0 = int(os.environ.get("CAPS_WARM_PASS0", "0"))


def _consts_np():
    """cstb [128,1024] bf16: BDF4 [0:512), BDT [512:1024).
    cstf [128,65] f32: masked-ones col 0; ones-row (partition 0) cols [1:65)."""
    cstb = np.zeros((128, 1024), dtype=np.float32)
    p = np.arange(128)
    # BDF4_j[p=(q,i), m] = 1 iff m == 32j + p//8  (i-reduce into band 32j+q)
    for j in range(4):
        cstb[p, 128 * j + 32 * j + p // 8] = 1.0
    # BDT_j = BDF4_j^T (band (j,q) -> rows (q,i))
    for j in range(4):
        cstb[:, 512 + 128 * j:512 + 128 * (j + 1)] = \
            cstb[:, 128 * j:128 * (j + 1)].T
    cstf = np.zeros((128, 65), dtype=np.float32)
    # Z-reduce mask: only band rows 32j+q (q<16) hold real data; the other
    # 64 partitions of wexpb are exp(0)=1 junk and must not enter Z.
    cstf[p[(p % 32) < 16], 0] = 1.0
    cstf[0, 1:65] = 1.0
    return cstb, cstf


def build_bass():
    f32 = mybir.dt.float32
    cdt = mybir.dt.bfloat16
    f8 = mybir.dt.float8e4

    nc = bacc.Bacc()
    # wxt: 8 chunks of [wk 2048 | xt 1024] columns
    wxt_d = nc.declare_dram_parameter("wxt", [128, 8 * 3072], cdt, isOutput=False)
    # xn8: fp8 x, natural layout on partitions 0:64
    # xn8: fp8 x, natural layout on partitions 0:64 (upper half zeroed on-chip)
    xn8_d = nc.declare_dram_parameter("xn8", [64, NT * 128], f8, isOutput=False)
    # xt8: fp8 x in (q,i)-partition layout for pass0
    xt8_d = nc.declare_dram_parameter("xt8", [128, NT * 64], f8, isOutput=False)
    cstb_d = nc.declare_dram_parameter("cstb", [128, 1024], cdt, isOutput=False)
    cstf_d = nc.declare_dram_parameter("cstf", [128, 65], f32, isOutput=False)
    out_d = nc.declare_dram_parameter("out", [B, F], f32, isOutput=True)

    with TileContext(nc) as tc, ExitStack() as ctx:
        big = ctx.enter_context(tc.tile_pool(name="big", bufs=1))
        small = ctx.enter_context(tc.tile_pool(name="small", bufs=3))
        pgpool = ctx.enter_context(tc.tile_pool(name="pgpool", bufs=3))
        p16 = ctx.enter_context(tc.tile_pool(name="p16", bufs=NB + 1))
        wwpool = ctx.enter_context(tc.tile_pool(name="wwpool", bufs=4))
        ps_acc = ctx.enter_context(tc.tile_pool(name="ps_acc", bufs=1, space="PSUM"))
        ps_gb = ctx.enter_context(tc.tile_pool(name="ps_gb", bufs=3, space="PSUM"))
        ps_misc = ctx.enter_context(tc.tile_pool(name="ps_misc", bufs=1, space="PSUM"))

        # ---- load inputs (consts first: small and needed early) ----
        cstb = big.tile([128, 1024], cdt, tag="cstb", name="cstb")
        nc.sync.dma_start(out=cstb, in_=cstb_d[:])
        cstf = big.tile([128, 65], f32, tag="cstf", name="cstf")
        nc.sync.dma_start(out=cstf, in_=cstf_d[:])
        xt8 = big.tile([128, NT * 64], f8, tag="xt8", name="xt8")
        for piece in range(2):
            c0 = piece * 4096
            nc.sync.dma_start(out=xt8[:, c0:c0 + 4096],
                              in_=xt8_d[:, c0:c0 + 4096])
        wxt = [big.tile([128, 3072], cdt, tag=f"wxt{h}", name=f"wxt{h}")
               for h in range(8)]
        xn8 = big.tile([128, NT * 128], f8, tag="xn8", name="xn8")
        nc.gpsimd.memset(xn8[64:128, :], 0.0)
        for h in range(8):
            nc.sync.dma_start(out=wxt[h][:, 0:2048],
                              in_=wxt_d[:, h * 3072:h * 3072 + 2048])
            if h == 2:
                for piece in range(2):
                    c0 = piece * 8192
                    nc.sync.dma_start(out=xn8[0:64, c0:c0 + 8192],
                                      in_=xn8_d[:, c0:c0 + 8192])
        for h in range(8):
            nc.sync.dma_start(out=wxt[h][:, 2048:3072],
                              in_=wxt_d[:, h * 3072 + 2048:(h + 1) * 3072])

        BDF4 = cstb[:, 0:512]
        BDT = cstb[:, 512:1024]
        onesm = cstf[:, 0:1]
        onesrow = cstf[0:1, 1:65]

        def wk_tile(t):
            h, lo = t // 16, t % 16
            return wxt[h][:, lo * 128:(lo + 1) * 128]

        def xt_tile(t):
            h, lo = t // 16, t % 16
            return wxt[h][:, 2048 + lo * 64:2048 + (lo + 1) * 64]

        def wk_block(hb):
            # [128, 8, 128] view of block hb's 8 k-tiles of W
            wkh = wxt[hb // 2][:, 0:2048].rearrange("p (u f) -> p u f", f=128)
            return wkh[:, (hb % 2) * BLK:(hb % 2) * BLK + BLK, :]

        # V: [128,128] bf16, upper half permanently zero so G matmuls can
        # run full-K (keeps the PE activity monitor at full clock)
        Vz = big.tile([128, 128], cdt, tag="Vz", name="Vz")
        nc.gpsimd.memset(Vz[64:128, :], 0.0)

        # v = n*|n| / (zsq + n^2); writes V into Vz[0:64] (mk_V) or returns out
        def squash_from(ps_n, zsq_sb, mk_V):
            absn = small.tile([64, 128], f32, tag="absn", name="absn")
            nc.scalar.activation(absn, ps_n, mybir.ActivationFunctionType.Abs)
            nsq = small.tile([64, 128], f32, tag="nsq", name="nsq")
            nc.scalar.activation(nsq, ps_n, mybir.ActivationFunctionType.Square)
            den = small.tile([64, 128], f32, tag="den", name="den")
            if zsq_sb is None:
                nc.vector.tensor_scalar_add(den, nsq, float(R) * float(R))
            else:
                nc.vector.tensor_add(den, nsq, zsq_sb)
            rden = small.tile([64, 128], f32, tag="rden", name="rden")
            nc.vector.reciprocal_approx_fast(rden, den)
            num = small.tile([64, 128], f32, tag="num", name="num")
            nc.vector.tensor_mul(num, ps_n, absn)
            if not mk_V:
                out_sb = small.tile([64, 128], f32, tag="outsb", name="outsb")
                nc.vector.tensor_mul(out_sb, num, rden)
                return out_sb
            nc.vector.tensor_mul(Vz[0:64, :], num, rden)
            return None

        # HAM warmup: long-stream dummy matmuls into a scratch PSUM tile to
        # keep the PE array's activity monitor at full clock across stalls.
        def warm(n, rhs):
            if n <= 0:
                return
            dmy = ps_gb.tile([128, 512], f32, tag="gb", name="warm")
            for _ in range(n):
                nc.tensor.matmul(dmy, lhsT=cstb[:, 0:128], rhs=rhs,
                                 start=True, stop=True)

        # ---- pass 0: n0 = sum_t xt8_t^T @ wk_t ; V = squash ----
        warm(8, cstb[:, 0:512])
        ps_s = ps_acc.tile([64, 128], f32, tag="acc", name="acc")
        for t in range(NT):
            nc.tensor.matmul(ps_s, lhsT=xt8[:, t * 64:(t + 1) * 64],
                             rhs=wk_tile(t),
                             start=(t == 0), stop=(t == NT - 1))
            if WARM_PASS0 and t % 16 == 15 and t // 16 < 7:
                warm(WARM_PASS0, wxt[t // 16][:, 0:512])
        squash_from(ps_s, None, True)
        warm(WARM_BOUND, wxt[7][:, 0:512])

        bstate = small.tile([128, 256], f32, tag="bstate", name="bstate", bufs=1)
        nc.vector.memset(bstate, 0.0)
        wexpb = small.tile([128, 256], cdt, tag="wexpb", name="wexpb", bufs=1)

        for it in (1, 2):
            ps_n = ps_acc.tile([64, 128], f32, tag="acc", name="acc")
            Ps = [None] * NB
            # ---------- phase A: all G matmuls (fp8 stationary x) ----------
            for hb in range(NB):
                psg = ps_gb.tile([128, BLK * 128], f32, tag="gb", name="gb")
                for lo in range(BLK):
                    t = hb * BLK + lo
                    nc.tensor.matmul(
                        psg[:, lo * 128:(lo + 1) * 128],
                        lhsT=xn8[:, t * 128:(t + 1) * 128], rhs=Vz,
                        start=True, stop=True,
                    )
                P = p16.tile([128, BLK * 128], cdt, tag="P", name="P")
                if hb in DIRECT_SET and (hb - 1) // 4 < DIRECT:
                    nc.vector.tensor_tensor(
                        P.rearrange("p (u f) -> p u f", f=128),
                        psg.rearrange("p (u f) -> p u f", f=128),
                        wk_block(hb),
                        op=mybir.AluOpType.mult,
                    )
                else:
                    Pg = pgpool.tile([128, BLK * 128], cdt, tag="Pg", name="Pg")
                    nc.scalar.activation(Pg, psg,
                                         mybir.ActivationFunctionType.Copy)
                    eng = nc.gpsimd if (hb in P_GPS_SET and
                                        len(P_GPS_SET) - list(sorted(P_GPS_SET)).index(hb) <= GPS_P) \
                        else nc.vector
                    eng.tensor_tensor(
                        P.rearrange("p (u f) -> p u f", f=128),
                        Pg.rearrange("p (u f) -> p u f", f=128),
                        wk_block(hb),
                        op=mybir.AluOpType.mult,
                    )
                Ps[hb] = P

            # ---------- phase B: BD bands + b-update + wrep ----------
            psbs = [None] * 4
            wrs = [None] * NB

            def emit_bd(grp):
                psb = ps_gb.tile([128, BLK * 128], f32, tag="gb", name="gb")
                for j in range(4):
                    for half in range(2):
                        nc.tensor.matmul(
                            psb[:, half * 512:(half + 1) * 512],
                            lhsT=BDF4[:, 128 * j:128 * (j + 1)],
                            rhs=Ps[4 * grp + j][:, half * 512:(half + 1) * 512],
                            start=(j == 0), stop=(j == 3),
                        )
                psbs[grp] = psb

            def emit_bupdate(grp):
                ored = small.tile([128, 64], f32, tag="ored", name="ored",
                                  bufs=2)
                psb = psbs[grp]
                nc.vector.tensor_reduce(
                    ored.rearrange("p (l c) -> p l c", c=8),
                    bass.AP(tensor=psb.tensor, offset=psb.offset,
                            ap=[psb.ap[0], [128, 8], [1, 8], [8, 16]]),
                    axis=mybir.AxisListType.X,
                    op=mybir.AluOpType.add,
                )
                cs = slice(grp * 64, (grp + 1) * 64)
                nc.vector.scalar_tensor_tensor(bstate[:, cs], ored, 1.0 / B,
                                               bstate[:, cs],
                                               op0=mybir.AluOpType.mult,
                                               op1=mybir.AluOpType.add)
                nc.scalar.activation(wexpb[:, cs], bstate[:, cs],
                                     mybir.ActivationFunctionType.Exp)

            def emit_wrep(grp):
                cs = slice(grp * 64, (grp + 1) * 64)
                for j in range(4):
                    hb = 4 * grp + j
                    ps_wr = ps_misc.tile([128, 64], f32, tag="m", name="wrps")
                    nc.tensor.matmul(ps_wr, lhsT=BDT[:, 128 * j:128 * (j + 1)],
                                     rhs=wexpb[:, cs], start=True, stop=True)
                    wr = small.tile([128, 64], cdt, tag="wr", name="wr", bufs=5)
                    nc.scalar.activation(wr, ps_wr,
                                         mybir.ActivationFunctionType.Copy)
                    wrs[hb] = wr

            emit_bd(0)
            emit_bupdate(0)
            emit_bd(1)
            emit_bupdate(1)
            emit_wrep(0)
            emit_bd(2)
            emit_bupdate(2)
            emit_wrep(1)
            emit_bd(3)
            emit_bupdate(3)
            emit_wrep(2)
            emit_wrep(3)

            # ---------- phase D: WW + n-matmuls; Z-path in the middle ----------
            def emit_ww_n(hb):
                wr = wrs[hb]
                ww = wwpool.tile([128, BLK * 128], cdt, tag="ww", name="ww")
                in1 = bass.AP(tensor=wr.tensor, offset=wr.offset,
                              ap=[wr.ap[0], [8, 8], [0, 16], [1, 8]])
                eng = nc.gpsimd if (hb in WW_GPS_SET and
                                    list(sorted(WW_GPS_SET, reverse=True)).index(hb) < GPS_WW) \
                    else nc.vector
                eng.tensor_tensor(
                    ww.rearrange("p (l o c) -> p l o c", o=16, c=8),
                    wk_block(hb).rearrange("p l (o c) -> p l o c", c=8),
                    in1,
                    op=mybir.AluOpType.mult,
                )
                for lo in range(BLK):
                    t = hb * BLK + lo
                    nc.tensor.matmul(ps_n,
                                     lhsT=xt_tile(t),
                                     rhs=ww[:, lo * 128:(lo + 1) * 128],
                                     start=(t == 0), stop=(t == NT - 1))

            for hb in range(4):
                emit_ww_n(hb)
            # Z^2 per c, replicated to [64, 128] (overlaps n-matmuls)
            wsum = small.tile([128, 8], f32, tag="wsum", name="wsum")
            nc.vector.tensor_reduce(
                wsum,
                bass.AP(tensor=wexpb.tensor, offset=wexpb.offset,
                        ap=[wexpb.ap[0], [1, 8], [8, 32]]),
                axis=mybir.AxisListType.X, op=mybir.AluOpType.add,
            )
            ps_z = ps_misc.tile([1, 8], f32, tag="m", name="zps")
            nc.tensor.matmul(ps_z, lhsT=onesm, rhs=wsum, start=True, stop=True)
            zsq = small.tile([1, 8], f32, tag="zsq", name="zsq")
            nc.scalar.activation(zsq, ps_z, mybir.ActivationFunctionType.Square)
            zrow = small.tile([1, 128], f32, tag="zrow", name="zrow")
            nc.scalar.activation(
                zrow.rearrange("p (o c) -> p o c", c=8),
                bass.AP(tensor=zsq.tensor, offset=zsq.offset,
                        ap=[zsq.ap[0], [0, 16], [1, 8]]),
                mybir.ActivationFunctionType.Copy,
            )
            ps_zq = ps_misc.tile([64, 128], f32, tag="m", name="zqps")
            nc.tensor.matmul(ps_zq, lhsT=onesrow, rhs=zrow, start=True, stop=True)
            zqsb = small.tile([64, 128], f32, tag="zqsb", name="zqsb")
            nc.scalar.activation(zqsb, ps_zq, mybir.ActivationFunctionType.Copy)
            for hb in range(4, NB):
                emit_ww_n(hb)

            if it < 2:
                squash_from(ps_n, zqsb, True)
                warm(WARM_BOUND, wxt[7][:, 0:512])
            else:
                out_sb = squash_from(ps_n, zqsb, False)
                nc.sync.dma_start(out=out_d[:], in_=out_sb)

    nc.finalize()
    return nc


def _host_prep(x, W):
    """Build per-core input dicts."""
    import ml_dtypes
    ct = ml_dtypes.bfloat16
    f8 = ml_dtypes.float8_e4m3fn
    x = np.ascontiguousarray(x, dtype=np.float32)
    W = np.ascontiguousarray(W, dtype=np.float32)
    # xt[p=(q,i), t*64+b] = x[b, 16t+q, i]
    xt = x.reshape(B, NT, 16, I).transpose(2, 3, 1, 0).reshape(128, NT, 64)
    # xn8[b, t*128 + q*8+i] = x[b, 16t+q, i]  (fp8, partitions 0..63)
    xn8 = x.reshape(B, NT * 128)
    cstb, cstf = _consts_np()
    in_maps = []
    for k in range(N_CORES):
        Ws = W[:, k * CL:(k + 1) * CL]  # [R, 8, O, I]
        wk = (Ws.reshape(NT, 16, CL, O, I).transpose(1, 4, 0, 3, 2)
              .reshape(128, NT, 128))
        wxt = np.zeros((128, 8, 3072), dtype=np.float32)
        for h in range(8):
            wxt[:, h, 0:2048] = wk[:, 16 * h:16 * (h + 1), :].reshape(128, 2048)
            wxt[:, h, 2048:3072] = xt[:, 16 * h:16 * (h + 1), :].reshape(128, 1024)
        in_maps.append({
            "wxt": np.ascontiguousarray(wxt.reshape(128, 8 * 3072), dtype=ct),
            "xn8": xn8.astype(f8),
            "xt8": np.ascontiguousarray(xt.reshape(128, NT * 64)).astype(f8),
            "cstb": cstb.astype(ct),
            "cstf": cstf,
        })
    return in_maps


_CACHE = {}


def _get_nc():
    if "nc" not in _CACHE:
        _CACHE["nc"] = build_bass()
    return _CACHE["nc"]


def run(x, W, trace=False):
    nc = _get_nc()
    in_maps = _host_prep(x, W)
    res = run_bass_kernel_spmd(nc, in_maps, core_ids=list(range(N_CORES)),
                               trace=trace)
    outs = [np.asarray(res.results[k]["out"], dtype=np.float32)
            for k in range(N_CORES)]
    # out[b, (o, c)]: core k holds capsules [8k, 8k+8)
    v = np.concatenate(
        [o.reshape(B, O, CL).transpose(0, 2, 1) for o in outs], axis=1)
    return v[..., None], res


def kernel(x, W):
    v, _ = run(np.asarray(x), np.asarray(W))
    return v
